# revision 1
# baseline (speedup 1.0000x reference)
"""Trainium2 Bass kernel for nn_NeuronAttention (moe_routing).

Sharding: data-parallel over batch B=8 across 8 NeuronCores (one batch row
per core); weights replicated; no collectives.

Per-core computation uses two layouts: "T-domain" [feature, token] for PE
GEMMs (contraction on partitions) and "N-domain" [token, small-free] for
routing math (softmax, top-k, Householder-chain recursion) on DVE/ACT.
The Householder chains are evaluated in 64-dim dot-space: with
d0 = xc@P.T, G = P@P.T, a = 1/(||P_k||^2+EPS), the 4 selected reflections
reduce to the scalar recursion beta_i = 2a_i(d0_i - sum_{j<i} beta_j G_ij)
and a rank-64 correction xc - (sum_i beta_i e_{idx_i})@P.

Precision: true fp32 (PE 4-pass) on every GEMM feeding router scores
(top-k gaps in this data go down to ~2e-6; FP22/bf16 would flip
selections), float32r (1-pass) only for the final output GEMM.
"""

import numpy as np

B, S, D, R = 8, 1024, 1024, 512
NPROC, TOPK = 64, 4
H, DH = 8, 64
EPS = 1e-8
NCORES = 8
TCN = 8   # token chunks of 128
KCN = 8   # D chunks of 128
RCN = 4   # rank chunks of 128
DEBUG = False
F32R_OUT = False
PHASES = 4
SUBB = 3
ROUTES = 3

_BUILT = {}


def _finish(nc):
    # sim-only partial builds skip the walrus wait-split
    return nc


def _apply_tile_drain_patch():
    """walrus here rejects >1 sync-wait on CTRL-class instructions; split
    Tile's kernel-tail drain waits into a chain of single-wait nops."""
    import concourse.mybir as mybir
    from concourse.tile import TileContext
    from concourse.vector_clock import ScopedClock

    if getattr(TileContext, "_drain_patched", False):
        return

    def _patched(self, tick_clock, wait_clock):
        probe = self.nc.sync.nop()
        wait_clock.add_sem_waits(
            probe.ins, ScopedClock({None: tick_clock.global_clock}))
        si = probe.ins.sync_info
        waits = list(si.on_wait) if si is not None else []
        updates = list(si.on_update) if si is not None else []
        if len(waits) > 1:
            probe.ins.sync_info = mybir.SyncInfo(
                on_update=updates, on_wait=waits[:1])
            for ofs in range(1, len(waits)):
                extra = self.nc.sync.nop()
                extra.ins.sync_info = mybir.SyncInfo(
                    on_update=[], on_wait=waits[ofs:ofs + 1])
        self.nc.sync.drain()
        self.nc.all_engine_barrier()
        assert self.sems is not None
        popped = self.nc._tile_sem_poison_stack.pop()
        assert popped is self._sem_poison
        self.nc.clear_and_free_semaphores(list(self.sems.allocated().values()))
        self.nc.all_engine_barrier()

    TileContext._drain_and_barrier = _patched
    TileContext._drain_patched = True


def _split_sync_waits(nc):
    """walrus here accepts at most 1 sync-wait per instruction; hoist
    extra waits onto same-engine NoOps inserted just before."""
    import concourse.mybir as mybir

    ctr = [0]
    for f in nc.m.functions:
        for bb in f.blocks:
            insts = bb.instructions
            out = []
            for inst in insts:
                si = inst.sync_info
                if si is not None and len(si.on_wait) > 1:
                    waits = list(si.on_wait)
                    for w in waits[:-1]:
                        ctr[0] += 1
                        nop = mybir.InstNoOp(
                            name=f"I-sw{ctr[0]}", ins=[], outs=[])
                        nop.engine = inst.engine
                        nop.sync_info = mybir.SyncInfo(
                            on_update=[], on_wait=[w])
                        out.append(nop)
                    inst.sync_info = mybir.SyncInfo(
                        on_update=list(si.on_update), on_wait=[waits[-1]])
                out.append(inst)
            bb.instructions = out


def _pack_kc(a, nchunk, chunk):
    # [nchunk*chunk, N] -> [chunk, nchunk*N], chunk-major partitions
    n = a.shape[1]
    return np.ascontiguousarray(
        a.reshape(nchunk, chunk, n).transpose(1, 0, 2).reshape(chunk, nchunk * n)
    ).astype(np.float32)


def prep_weights(inputs):
    f = {k: np.asarray(v, np.float64) for k, v in inputs.items()}
    P = f["process_hh"]
    G = P @ P.T
    alpha2 = 2.0 / ((P * P).sum(1) + EPS)
    ihh, ohh = f["input_hh"], f["output_hh"]
    base_in, base_out = f["base_input"], f["base_output"]
    Bo = ohh @ base_out.T

    w = {}
    w["BI"] = _pack_kc(base_in, KCN, 128)
    W4 = np.concatenate([f["q_in_router"].T, f["k_in_router"].T,
                         f["v_in_router"].T, ihh.T], axis=1)
    w["W4"] = _pack_kc(W4, KCN, 128)
    for nm, wp in (("WDRQ", "q_proc_router"), ("WDRK", "k_proc_router"),
                   ("WDRV", "v_proc_router"), ("WDRO", "o_proc_router")):
        w[nm] = _pack_kc(np.concatenate([P.T, f[wp].T], axis=1), RCN, 128)
    w["WDRO2"] = _pack_kc(
        np.concatenate([f["o_out_router"].T, Bo.T,
                        np.zeros((512, 128))], axis=1), RCN, 128)
    w["NEGBH"] = (-(ihh @ base_in)).astype(np.float32)
    w["NEGP"] = (-P).astype(np.float32)
    w["GIN"] = (ihh @ ihh.T).astype(np.float32)
    w["GOUT"] = (ohh @ ohh.T).astype(np.float32)
    BD = np.zeros((256, 196))
    for i in range(4):
        if i < 3:
            BD[64 * i:64 * i + 64, 64 * i:64 * i + 64] = -G
        BD[64 * i:64 * i + 64, 192 + i] = alpha2
    w["BD"] = _pack_kc(BD, 2, 128)
    w["BOUT"] = _pack_kc(base_out, RCN, 128)
    w["NEGOHH"] = (-ohh).astype(np.float32)
    w["NEGPBO"] = (-(P @ base_out)).astype(np.float32)
    w["NEGPOW"] = (-(P @ np.concatenate(
        [f["o_out_router"].T, Bo.T], axis=1))).astype(np.float32)
    w["NPBOHH"] = np.concatenate(
        [-(P @ base_out), -ohh], axis=0).astype(np.float32)
    return w


def build():
    import concourse.bass as bass
    import concourse.mybir as mybir
    from concourse.tile import TileContext
    from concourse.masks import make_identity

    _apply_tile_drain_patch()
    dt = mybir.dt
    op = mybir.AluOpType
    act = mybir.ActivationFunctionType

    nc = bass.Bass()
    XTd = nc.dram_tensor("XT", (128, KCN * 1024), dt.float32, kind="ExternalInput")
    wd = {}
    for nm, shape in (
        ("BI", (128, KCN * 512)), ("W4", (128, KCN * 256)),
        ("WDRQ", (128, RCN * 128)), ("WDRK", (128, RCN * 128)),
        ("WDRV", (128, RCN * 128)), ("WDRO", (128, RCN * 128)),
        ("WDRO2", (128, RCN * 256)),
        ("NEGBH", (64, 512)), ("NEGP", (64, 512)),
        ("GIN", (64, 64)), ("GOUT", (64, 64)),
        ("BD", (128, 2 * 196)), ("BOUT", (128, RCN * 1024)),
        ("NEGOHH", (64, 1024)), ("NEGPBO", (64, 1024)), ("NEGPOW", (64, 128)), ("NPBOHH", (128, 1024)),
    ):
        wd[nm] = nc.dram_tensor(nm, shape, dt.float32, kind="ExternalInput")
    OUTd = nc.dram_tensor("OUT", (1024, 1024), dt.float32, kind="ExternalOutput")
    dbg = {}
    if DEBUG:
        for nm, shape in (
            ("DSF", (8, 128, 256)), ("DXB", (4, 128, 1024)),
            ("DXCQ", (4, 128, 1024)), ("DRQ", (128, 8, 64)),
            ("DD0Q", (128, 8, 64)), ("DM8Q", (128, 8, 8)),
            ("DCQ", (128, 8, 64)), ("DQT", (4, 128, 1024)),
            ("DVP", (8, 128, 520)), ("DAO", (8, 128, 512)),
            ("DX4", (4, 128, 1024)), ("DBETAQ", (128, 8, 4)),
        ):
            dbg[nm] = nc.dram_tensor(nm, shape, dt.float32, kind="ExternalOutput")

    f32r = dt.float32r

    with TileContext(nc) as tc:
        with (
            tc.tile_pool(name="w", bufs=1) as pw,
            tc.tile_pool(name="live", bufs=1) as pl,
        ):
            W = {}
            for nm, dram in wd.items():
                if nm in ("BOUT", "NEGOHH", "WDRO2", "NEGPBO", "NEGPOW", "NPBOHH"):
                    continue
                t = pw.tile(list(dram.shape), dt.float32, tag=nm)
                nc.sync.dma_start(out=t[:], in_=dram[:])
                W[nm] = t
            ident = pw.tile([128, 128], dt.float32, tag="ident", name="ident")
            make_identity(nc, ident[:])

            BI = W["BI"][:].rearrange("p (k n) -> p k n", k=KCN)
            W4 = W["W4"][:].rearrange("p (k n) -> p k n", k=KCN)
            BD = W["BD"][:].rearrange("p (k n) -> p k n", k=2)
            WDR = {r: W["WDR" + r][:].rearrange("p (k n) -> p k n", k=RCN)
                   for r in ("Q", "K", "V", "O")}
            XTv = XTd[:].rearrange("p (k n) -> p k n", k=KCN)

            # persistent activations
            AOT = [pl.tile([128, 1024], dt.float32, tag=f"aot{rc}", name=f"aot{rc}")
                   for rc in range(RCN)]

            # ---------- shared helpers ----------

            def dot64(acc, a, b, pool, nm):
                scr = pool.tile([128, 64], dt.float32, tag="scr_sh",
                                name=f"scr_{nm}", bufs=4)
                nc.vector.tensor_mul(scr[:], a, b)
                nc.vector.tensor_reduce(acc, scr[:], mybir.AxisListType.X,
                                        op.add)
            def softmax_front(pool, ppt, ppv, name, s_all, f_all, gram,
                              cht_dtype=None, cht_ap=None):
                """Batched over all 8 token chunks. s_all/f_all are
                [128, 8, 64] APs. Returns CHT [64, 1024] (chat^T)."""
                E = pool.tile([128, TCN * 64], dt.float32, tag="E_sh",
                              name=f"E_{name}", bufs=2)
                Ev = E[:].rearrange("p (t n) -> p t n", t=TCN)
                ET = pool.tile([64, 1024], dt.float32, tag="ET_sh",
                               name=f"ET_{name}", bufs=1)
                CH = pool.tile([128, TCN * 64], dt.float32, tag="CH_sh",
                               name=f"CH_{name}", bufs=2)
                CHv = CH[:].rearrange("p (t n) -> p t n", t=TCN)
                CHT = cht_ap if cht_ap is not None else pool.tile(
                    [64, 1024], cht_dtype or dt.float32,
                    tag="CHT_sh", name=f"CHT_{name}", bufs=1)
                SC = pool.tile([128, 8 * 8], dt.float32, tag=f"sc1_{name}", name=f"sc1_{name}")
                SCv = SC[:].rearrange("p (t n) -> p t n", t=8)
                scr = pool.tile([128, 512], dt.float32, tag="scr_sh",
                                name=f"scr_{name}", bufs=4)

                nc.scalar.activation(Ev[:, :, :], s_all, act.Exp)
                Z8 = SCv[:, :, 0:1]
                nc.vector.tensor_reduce(Z8, Ev[:, :, :],
                                        mybir.AxisListType.X, op.add)
                # u = e @ Gin per chunk, packed into one PSUM bank
                pu = ppv.tile([128, 512], dt.float32, tag="ps_u", name="ps_u")
                for t in range(TCN):
                    pt = ppt.tile([128, 128], dt.float32, tag="ps_t", name="ps_t")
                    nc.tensor.transpose(pt[0:64, :], Ev[:, t, :], ident[:])
                    nc.scalar.copy(ET[:, 128 * t:128 * (t + 1)], pt[0:64, :])
                    nc.tensor.matmul(pu[:, 64 * t:64 * (t + 1)],
                                     ET[:, 128 * t:128 * (t + 1)], gram,
                                     start=True, stop=True)
                puv = pu[:].rearrange("p (t n) -> p t n", t=TCN)
                pacc, qacc = SCv[:, :, 1:2], SCv[:, :, 2:3]
                nc.vector.tensor_mul(scr[:], Ev[:, :, :], f_all)
                nc.vector.tensor_reduce(
                    pacc, scr[:].rearrange("p (t n) -> p t n", t=TCN),
                    mybir.AxisListType.X, op.add)
                nc.vector.tensor_mul(scr[:], Ev[:, :, :], puv)
                nc.vector.tensor_reduce(
                    qacc, scr[:].rearrange("p (t n) -> p t n", t=TCN),
                    mybir.AxisListType.X, op.add)
                z2, den = SCv[:, :, 3:4], SCv[:, :, 4:5]
                rec, gam = SCv[:, :, 5:6], SCv[:, :, 6:7]
                nc.vector.tensor_mul(z2, Z8, Z8)
                nc.vector.scalar_tensor_tensor(out=den, in0=z2, scalar=EPS,
                                               in1=qacc, op0=op.mult, op1=op.add)
                nc.vector.reciprocal(rec, den)
                nc.vector.scalar_tensor_tensor(out=gam, in0=pacc, scalar=2.0,
                                               in1=rec, op0=op.mult, op1=op.mult)
                nc.vector.tensor_mul(CHv[:, :, :], Ev[:, :, :],
                                     gam.to_broadcast((128, TCN, 64)))
                for t in range(TCN):
                    pt2 = ppt.tile([128, 128], dt.float32, tag="ps_t", name="ps_t")
                    nc.tensor.transpose(pt2[0:64, :], CHv[:, t, :], ident[:])
                    nc.scalar.copy(CHT[:, 128 * t:128 * (t + 1)], pt2[0:64, :])
                return CHT

            def hh_chain_a(pool, pps, ppt, name, wdr, src_tiles,
                           dbg_pfx=None, ct_ap=None):
                """Stage a: D0/R GEMM, top-4 one-hots, d0 selects, -G row
                gather. Returns tile dict for hh_chain_b."""
                D0 = pool.tile([128, TCN * 64], dt.float32, tag="D0_sh",
                               name=f"D0_{name}", bufs=2)
                D0v = D0[:].rearrange("p (t n) -> p t n", t=TCN)
                RS = pool.tile([128, TCN * 64], dt.float32, tag="RS_sh",
                               name=f"RS_{name}", bufs=2)
                RSv = RS[:].rearrange("p (t n) -> p t n", t=TCN)
                M8 = pool.tile([128, TCN * 8], dt.float32, tag="M8_sh",
                               name=f"M8_{name}", bufs=1)
                M8v = M8[:].rearrange("p (t n) -> p t n", t=TCN)
                OH = pool.tile([128, TCN * 256], dt.float32, tag="OH_sh",
                               name=f"OH_{name}", bufs=2)
                OHv = OH[:].rearrange("p (t n) -> p t n", t=TCN)
                OHT = pool.tile([128, 256], dt.float32, tag="OHT_sh",
                                name=f"OHT_{name}", bufs=2)
                BT = pool.tile([128, TCN * 196], dt.float32, tag="BT_sh",
                               name=f"BT_{name}", bufs=2)
                BTv = BT[:].rearrange("p (t n) -> p t n", t=TCN)
                DSA = pool.tile([128, TCN * 24], dt.float32, tag="DS_sh2",
                                name=f"DS_{name}", bufs=2)
                DSAv = DSA[:].rearrange("p (t n) -> p t n", t=TCN)
                BE = pool.tile([128, TCN * 4], dt.float32, tag=f"BE_{name}", name=f"BE_{name}")
                BEv = BE[:].rearrange("p (t n) -> p t n", t=TCN)
                CC = pool.tile([128, TCN * 64], dt.float32, tag="CC_sh",
                               name=f"CC_{name}", bufs=2)
                CCv = CC[:].rearrange("p (t n) -> p t n", t=TCN)
                CT = ct_ap if ct_ap is not None else pool.tile(
                    [64, 1024], dt.float32, tag="CT_sh",
                    name=f"CT_{name}", bufs=2)
                scr = pool.tile([128, 512], dt.float32, tag="scr_sh",
                                name=f"scr_{name}2", bufs=4)
                scrv = scr[:].rearrange("p (t n) -> p t n", t=TCN)

                # D0/R: pack 4 chunks per PSUM bank
                psd = [pps.tile([128, 512], dt.float32, tag="ps_sf",
                                name="ps_sf", bufs=2)
                       for _ in range(2)]
                for t in range(TCN):
                    for rc in range(RCN):
                        nc.tensor.matmul(
                            psd[t // 4][:, 128 * (t % 4):128 * (t % 4 + 1)],
                            src_tiles[rc][:, 128 * t:128 * (t + 1)],
                            wdr[:, rc, :],
                            start=(rc == 0), stop=(rc == RCN - 1))
                for half in range(2):
                    pv = psd[half][:].rearrange("p (t n) -> p t n", t=4)
                    nc.scalar.copy(D0v[:, 4 * half:4 * half + 4, :],
                                   pv[:, :, 0:64])
                    nc.scalar.copy(RSv[:, 4 * half:4 * half + 4, :],
                                   pv[:, :, 64:128])
                for t in range(TCN):
                    nc.vector.max(M8v[:, t, :], RSv[:, t, :])
                # one-hots + d0 selects (batched over chunks)
                for i in range(4):
                    nc.vector.tensor_tensor(
                        OHv[:, :, 64 * i:64 * (i + 1)], RSv[:, :, :],
                        M8v[:, :, i:i + 1].to_broadcast((128, TCN, 64)),
                        op.is_equal)
                    nc.vector.tensor_mul(scr[:], OHv[:, :, 64 * i:64 * (i + 1)],
                                         D0v[:, :, :])
                    nc.vector.tensor_reduce(DSAv[:, :, i:i + 1], scrv,
                                            mybir.AxisListType.X, op.add)
                # gather -G rows + 2alpha via transposed one-hot GEMM
                for t in range(TCN):
                    psb = pps.tile([128, 196], dt.float32, tag="ps_mm",
                                   name="ps_b", bufs=3)
                    for half in range(2):
                        pt = ppt.tile([128, 128], dt.float32, tag="ps_t", name="ps_t")
                        nc.tensor.transpose(
                            pt[:], OHv[:, t, 128 * half:128 * (half + 1)],
                            ident[:])
                        nc.scalar.copy(OHT[:, 128 * half:128 * (half + 1)], pt[:])
                        nc.tensor.matmul(
                            psb[:], OHT[:, 128 * half:128 * (half + 1)],
                            BD[:, half, :], start=(half == 0), stop=(half == 1))
                    nc.scalar.copy(BTv[:, t, :], psb[:])
                if DEBUG and dbg_pfx:
                    nc.sync.dma_start(out=dbg[f"DR{dbg_pfx}"][:], in_=RSv[:, :, :])
                    nc.sync.dma_start(out=dbg[f"DD0{dbg_pfx}"][:], in_=D0v[:, :, :])
                    nc.sync.dma_start(out=dbg[f"DM8{dbg_pfx}"][:], in_=M8v[:, :, :])
                return dict(OHv=OHv, BTv=BTv, DSAv=DSAv, BEv=BEv,
                            CCv=CCv, CC=CC, CT=CT, scr=scr, scrv=scrv)

            def hh_chain_b(st):
                OHv, BTv, DSAv = st["OHv"], st["BTv"], st["DSAv"]
                BEv, CCv, CC = st["BEv"], st["CCv"], st["CC"]
                CT, scr, scrv = st["CT"], st["scr"], st["scrv"]
                # pair values -G[idx_i, idx_j]
                pair = {}
                pidx = 4
                for i in range(1, 4):
                    for j in range(i):
                        nc.vector.tensor_mul(
                            scr[:], OHv[:, :, 64 * i:64 * (i + 1)],
                            BTv[:, :, 64 * j:64 * (j + 1)])
                        nc.vector.tensor_reduce(
                            DSAv[:, :, pidx:pidx + 1], scrv,
                            mybir.AxisListType.X, op.add)
                        pair[(i, j)] = DSAv[:, :, pidx:pidx + 1]
                        pidx += 1
                # recursion (batched [128, 8] ops)
                be = [BEv[:, :, i:i + 1] for i in range(4)]
                a2 = [BTv[:, :, 192 + i:193 + i] for i in range(4)]
                nc.vector.tensor_mul(be[0], DSAv[:, :, 0:1], a2[0])
                tmp = 10
                for i in range(1, 4):
                    cur = DSAv[:, :, i:i + 1]
                    for j in range(i):
                        t1 = DSAv[:, :, tmp:tmp + 1]; tmp += 1
                        nc.vector.tensor_mul(t1, pair[(i, j)], be[j])
                        t2 = DSAv[:, :, tmp:tmp + 1]; tmp += 1
                        nc.vector.tensor_add(t2, t1, cur)
                        cur = t2
                    nc.vector.tensor_mul(be[i], cur, a2[i])
                # c = sum beta_i * onehot_i
                nc.vector.tensor_mul(CCv[:, :, :], OHv[:, :, 0:64],
                                     be[0].to_broadcast((128, TCN, 64)))
                for i in range(1, 4):
                    nc.vector.tensor_mul(
                        scr[:], OHv[:, :, 64 * i:64 * (i + 1)],
                        be[i].to_broadcast((128, TCN, 64)))
                    nc.vector.tensor_add(CC[:], CC[:], scr[:])
                for t in range(TCN):
                    ptc = ppt.tile([128, 128], dt.float32, tag="ps_t", name="ps_t")
                    nc.tensor.transpose(ptc[0:64, :], CCv[:, t, :], ident[:])
                    nc.scalar.copy(CT[:, 128 * t:128 * (t + 1)], ptc[0:64, :])
                return CT

            def hh_chain(pool, pps, ppt, name, wdr, src_tiles,
                         dbg_pfx=None, ct_ap=None):
                st = hh_chain_a(pool, pps, ppt, name, wdr, src_tiles,
                                dbg_pfx, ct_ap=ct_ap)
                return hh_chain_b(st)

            def neg_corr(pps, lhs64, ct, dst_tiles, src_tiles):
                """dst = src + lhs64.T @ ct  (K=64 correction GEMM + add)."""
                for rc in range(RCN):
                    for th in range(2):
                        ps = pps.tile([128, 512], dt.float32, tag="ps_mm",
                                      name="ps_mm", bufs=3)
                        nc.tensor.matmul(
                            ps[:], lhs64[0:64, 128 * rc:128 * (rc + 1)],
                            ct[0:64, 512 * th:512 * (th + 1)],
                            start=True, stop=True)
                        nc.vector.tensor_add(
                            dst_tiles[rc][:, 512 * th:512 * (th + 1)],
                            src_tiles[rc][:, 512 * th:512 * (th + 1)], ps[:])

            # ================= phase A: SF + xbT =================
            _pbc_cm = tc.tile_pool(name="bc", bufs=1)
            pbc = _pbc_cm.__enter__()
            XC = {r: [pbc.tile([128, 1024], dt.float32, tag=f"xc{r}{rc}", name=f"xc{r}{rc}")
                      for rc in range(RCN)] for r in "qk"}
            VP = [pbc.tile([128, 520], dt.float32, tag=f"vp{kc}", name=f"vp{kc}")
                  for kc in range(KCN)]
            with tc.tile_pool(name="phb", bufs=1) as pb:
                XC["v"] = None  # aliased to XB below (in-place)
                SFt = pb.tile([128, TCN * 256], dt.float32, tag="BT_sh",
                              name="sf", bufs=2)
                SFv = SFt[:].rearrange("p (t n) -> p t n", t=TCN)
                XB = [pb.tile([128, 1024], dt.float32, tag=f"xb{rc}", name=f"xb{rc}")
                      for rc in range(RCN)]
                with (
                    tc.tile_pool(name="psA", bufs=4, space="PSUM") as psA,
                    tc.tile_pool(name="pxt", bufs=2) as px,
                ):
                    for sweep in range(2):
                        ps_sf = [psA.tile([128, 256], dt.float32, tag="ps_asf",
                                          name="ps_asf") for _ in range(4)]
                        ps_xb = [psA.tile([128, 512], dt.float32, tag="ps_axb",
                                          name="ps_axb") for _ in range(4)]
                        for kc in range(KCN):
                            xt = px.tile([128, 1024], dt.float32, tag="xtc",
                                         name="xtc")
                            nc.sync.dma_start(out=xt[:], in_=XTv[:, kc, :])
                            for ti in range(4):
                                nc.tensor.matmul(
                                    ps_sf[ti][:],
                                    xt[:, 128 * ti + 512 * sweep:
                                       128 * (ti + 1) + 512 * sweep],
                                    W4[:, kc, :],
                                    start=(kc == 0), stop=(kc == KCN - 1))
                            for i in range(4):
                                rc, th = 2 * sweep + i // 2, i % 2
                                nc.tensor.matmul(
                                    ps_xb[i][:],
                                    BI[:, kc, 128 * rc:128 * (rc + 1)],
                                    xt[:, 512 * th:512 * (th + 1)],
                                    start=(kc == 0), stop=(kc == KCN - 1))
                        for ti in range(4):
                            t = 4 * sweep + ti
                            nc.scalar.copy(SFv[:, t, :], ps_sf[ti][:])
                            if DEBUG:
                                nc.sync.dma_start(out=dbg["DSF"][t],
                                                  in_=SFv[:, t, :])
                        for i in range(4):
                            rc, th = 2 * sweep + i // 2, i % 2
                            nc.scalar.copy(
                                XB[rc][:, 512 * th:512 * (th + 1)], ps_xb[i][:])
                    if DEBUG:
                        for rc in range(RCN):
                            nc.sync.dma_start(out=dbg["DXB"][rc], in_=XB[rc][:])

                # ============ phase B: compress routes ============
                if PHASES < 2:
                    return _finish(nc)
                XC["v"] = XB
                with (
                    tc.tile_pool(name="psB", bufs=1, space="PSUM") as pps,
                    tc.tile_pool(name="psBt", bufs=2, space="PSUM") as ppt,
                    tc.tile_pool(name="psBv", bufs=1, space="PSUM") as ppv,
                ):
                    CTs = {}
                    for ri, r in enumerate("qkv"[:ROUTES]):
                        CHT = softmax_front(
                            pb, ppt, ppv, r,
                            SFv[:, :, 64 * ri:64 * ri + 64],
                            SFv[:, :, 192:256], W["GIN"][:])
                        neg_corr(pps, W["NEGBH"][:], CHT, XC[r], XB)
                        if DEBUG and r == "q":
                            for rc in range(RCN):
                                nc.sync.dma_start(out=dbg["DXCQ"][rc],
                                                  in_=XC[r][rc][:])
                    sts = {}
                    for ri, r in enumerate("qkv"[:ROUTES]):
                        if SUBB < 2:
                            continue
                        sts[r] = hh_chain_a(pb, pps, ppt, r, WDR["QKV"[ri]],
                                            XC[r],
                                            dbg_pfx=("Q" if r == "q" else None))
                    for ri, r in enumerate("qkv"[:ROUTES]):
                        if SUBB < 2 or SUBB < 3:
                            continue
                        CTs[r] = hh_chain_b(sts[r])
                        neg_corr(pps, W["NEGP"][:], CTs[r], XC[r], XC[r])
                    if DEBUG:
                        for rc in range(RCN):
                            nc.sync.dma_start(out=dbg["DQT"][rc],
                                              in_=XC["q"][rc][:])
                    # V -> N-domain V' with per-head 65-col blocks + ones
                    for rc in range(RCN if SUBB >= 3 else 0):
                        for t in range(TCN):
                            pt = ppt.tile([128, 128], dt.float32, tag="ps_t", name="ps_t")
                            nc.tensor.transpose(
                                pt[:], XC["v"][rc][:, 128 * t:128 * (t + 1)],
                                ident[:])
                            dst = bass.AP(
                                VP[t].tensor, VP[t].offset + 65 * (2 * rc),
                                [VP[t].ap[0], [65, 2], [1, 64]])
                            nc.scalar.copy(
                                dst, pt[:].rearrange("p (h n) -> p h n", h=2))
                    for t in range(TCN):
                        ones = VP[t][:].rearrange(
                            "p (h n) -> p h n", h=H)[:, :, 64:65]
                        nc.vector.memset(ones, 1.0)
                    if DEBUG:
                        for t in range(TCN):
                            nc.sync.dma_start(out=dbg["DVP"][t], in_=VP[t][:])

            # ================= phase C: attention =================
            if PHASES < 3:
                _pbc_cm.__exit__(None, None, None)
                return _finish(nc)
            with tc.tile_pool(name="att", bufs=1) as pa:
                AO = [pa.tile([128, 512], dt.float32, tag=f"ao{qt}", name=f"ao{qt}")
                      for qt in range(TCN)]
                def new_pt(hi):
                    t = pa.tile([128, KCN * 512], dt.float32, tag=f"pt{hi}",
                                name=f"pt{hi}", bufs=2)
                    return t[:].rearrange("p (k n) -> p k n", k=KCN)
                RSE = pa.tile([128, 8], dt.float32, tag="rse", name="rse")
                with (
                    tc.tile_pool(name="psC", bufs=3, space="PSUM") as pps,
                    tc.tile_pool(name="psCt", bufs=2, space="PSUM") as ppt,
                    tc.tile_pool(name="psCv", bufs=3, space="PSUM") as ppv,
                ):
                    for hp in range(4):
                        for qh in range(2):
                            for hi, h in enumerate((2 * hp, 2 * hp + 1)):
                                hr = 64 * hi
                                ptv = new_pt(hi)
                                for kc in range(KCN):
                                    ps = pps.tile([128, 512], dt.float32,
                                                  tag="ps_mm", name="ps_mm")
                                    nc.tensor.matmul(
                                        ps[:],
                                        XC["k"][hp][hr:hr + 64,
                                                    128 * kc:128 * (kc + 1)],
                                        XC["q"][hp][hr:hr + 64,
                                                    512 * qh:512 * (qh + 1)],
                                        start=True, stop=True)
                                    nc.scalar.activation(
                                        ptv[:, kc, :], ps[:], act.Exp,
                                        scale=0.125)
                                for qc in range(4):
                                    qt = 4 * qh + qc
                                    pv = ppv.tile([128, 65], dt.float32,
                                                  tag="ps_pv", name="ps_pv")
                                    for kc in range(KCN):
                                        nc.tensor.matmul(
                                            pv[:],
                                            ptv[:, kc,
                                                128 * qc:128 * (qc + 1)],
                                            VP[kc][:, 65 * h:65 * h + 65],
                                            start=(kc == 0),
                                            stop=(kc == KCN - 1))
                                    rse = RSE[:, h:h + 1]
                                    nc.vector.reciprocal(rse, pv[:, 64:65])
                                    nc.vector.tensor_scalar_mul(
                                        AO[qt][:, 64 * h:64 * (h + 1)],
                                        pv[:, 0:64], rse)
                    for rc in range(RCN):
                        for qt in range(TCN):
                            pt = ppt.tile([128, 128], dt.float32, tag="ps_t", name="ps_t")
                            nc.tensor.transpose(
                                pt[:], AO[qt][:, 128 * rc:128 * (rc + 1)],
                                ident[:])
                            nc.scalar.copy(
                                AOT[rc][:, 128 * qt:128 * (qt + 1)], pt[:])
                    if DEBUG:
                        for qt in range(TCN):
                            nc.sync.dma_start(out=dbg["DAO"][qt], in_=AO[qt][:])

            _pbc_cm.__exit__(None, None, None)
            # ================= phase D: expand =================
            if PHASES < 4:
                return _finish(nc)
            with tc.tile_pool(name="exp", bufs=1) as pe:
                xdt = f32r if F32R_OUT else dt.float32
                BOUTt = pe.tile([128, RCN * 1024], xdt, tag="boutw", name="boutw")
                nc.sync.dma_start(out=BOUTt[:], in_=wd["BOUT"][:].bitcast(xdt))
                BOUT = BOUTt[:].rearrange("p (k n) -> p k n", k=RCN)
                NEGOHHt = pe.tile([64, 1024], xdt, tag="negohhw", name="negohhw")
                nc.sync.dma_start(out=NEGOHHt[:], in_=wd["NEGOHH"][:].bitcast(xdt))
                WDRO2t = pe.tile([128, RCN * 256], xdt, tag="wdro2w", name="wdro2w")
                nc.sync.dma_start(out=WDRO2t[:], in_=wd["WDRO2"][:].bitcast(xdt))
                WDRO2 = WDRO2t[:].rearrange("p (k n) -> p k n", k=RCN)
                NEGPBOt = pe.tile([64, 1024], xdt, tag="negpbow", name="negpbow")
                nc.sync.dma_start(out=NEGPBOt[:], in_=wd["NEGPBO"][:].bitcast(xdt))
                NEGPOWt = pe.tile([64, 128], xdt, tag="negpoww", name="negpoww")
                nc.sync.dma_start(out=NEGPOWt[:], in_=wd["NEGPOW"][:].bitcast(xdt))
                NPBOHHt = pe.tile([128, 1024], xdt, tag="npbohhw", name="npbohhw")
                nc.sync.dma_start(out=NPBOHHt[:], in_=wd["NPBOHH"][:].bitcast(xdt))
                STK = pe.tile([128, 1024], xdt, tag="stk", name="stk")
                OUT1 = [pe.tile([128, 1024], dt.float32, tag=f"out1{dc}",
                                name=f"out1{dc}") for dc in range(KCN)]
                SO = pe.tile([128, TCN * 128], dt.float32, tag="so", name="so")
                SOv = SO[:].rearrange("p (t n) -> p t n", t=TCN)
                with (
                    tc.tile_pool(name="psD", bufs=1, space="PSUM") as pps,
                    tc.tile_pool(name="psDt", bufs=2, space="PSUM") as ppt,
                    tc.tile_pool(name="psDv", bufs=1, space="PSUM") as ppv,
                ):
                    CTo = hh_chain(pe, pps, ppt, "o", WDR["O"], AOT,
                                   ct_ap=STK[0:64, :])
                    so_n = 128
                    for t in range(TCN):
                        ps = pps.tile([128, so_n], dt.float32, tag="ps_sf",
                                      name="ps_sf", bufs=2)
                        for rc in range(RCN):
                            nc.tensor.matmul(
                                ps[:], AOT[rc][:, 128 * t:128 * (t + 1)],
                                WDRO2[:, rc, 0:so_n],
                                start=(rc == 0), stop=False)
                        nc.tensor.matmul(
                            ps[:], CTo[0:64, 128 * t:128 * (t + 1)],
                            NEGPOWt[0:64, :], start=False, stop=True)
                        nc.scalar.copy(SOv[:, t, :], ps[:, 0:128])
                    # ao @ base_out — emitted late so it back-fills PE stalls
                    for dc in range(KCN):
                        for th in range(2):
                            ps = pps.tile([128, 512], dt.float32, tag="ps_mm",
                                          name="ps_mm", bufs=3)
                            for rc in range(RCN):
                                nc.tensor.matmul(
                                    ps[:], BOUT[:, rc, 128 * dc:128 * (dc + 1)],
                                    AOT[rc][:, 512 * th:512 * (th + 1)],
                                    start=(rc == 0), stop=(rc == RCN - 1))
                            nc.scalar.copy(
                                OUT1[dc][:, 512 * th:512 * (th + 1)], ps[:])
                    CHoT = softmax_front(
                        pe, ppt, ppv, "o",
                        SOv[:, :, 0:64], SOv[:, :, 64:128], W["GOUT"][:],
                        cht_dtype=xdt, cht_ap=STK[64:128, :])
                    for dc in range(KCN):
                        for th in range(2):
                            ps = pps.tile([128, 512], dt.float32, tag="ps_mm",
                                          name="ps_mm", bufs=3)
                            nc.tensor.matmul(
                                ps[:], NPBOHHt[:, 128 * dc:128 * (dc + 1)],
                                STK[:, 512 * th:512 * (th + 1)],
                                start=True, stop=True)
                            ot = pe.tile([128, 512], dt.float32, tag="outsb",
                                         name="outsb", bufs=3)
                            nc.vector.tensor_add(
                                ot[:], OUT1[dc][:, 512 * th:512 * (th + 1)],
                                ps[:])
                            nc.sync.dma_start(
                                out=OUTd[128 * dc:128 * (dc + 1),
                                         512 * th:512 * (th + 1)],
                                in_=ot[:])
    _split_sync_waits(nc)
    return nc


def get_built():
    if "nc" not in _BUILT:
        _BUILT["nc"] = build()
    return _BUILT["nc"]


def kernel(**inputs):
    from concourse.bass_utils import run_bass_kernel_spmd

    x = np.asarray(inputs["x"], np.float32)
    w = prep_weights(inputs)
    nc = get_built()
    in_maps = []
    for c in range(NCORES):
        m = dict(w)
        m["XT"] = _pack_kc(np.ascontiguousarray(x[c].T), KCN, 128)
        in_maps.append(m)
    res = run_bass_kernel_spmd(nc, in_maps, core_ids=list(range(NCORES)))
    out = np.stack([res.results[c]["OUT"].T for c in range(NCORES)], axis=0)
    return out.astype(np.float32)


def run_timed(inputs, trace=False):
    from concourse.bass_utils import run_bass_kernel_spmd
    x = np.asarray(inputs["x"], np.float32)
    w = prep_weights(inputs)
    nc = get_built()
    in_maps = []
    for c in range(NCORES):
        m = dict(w)
        m["XT"] = _pack_kc(np.ascontiguousarray(x[c].T), KCN, 128)
        in_maps.append(m)
    return run_bass_kernel_spmd(nc, in_maps, core_ids=list(range(NCORES)),
                                trace=trace)

